# revision 8
# baseline (speedup 1.0000x reference)
"""Differential attention kernel for 8 Trainium2 NeuronCores.

Sharding: batch x head-group. Core c handles batch b = c//4, heads
[4g, 4g+4) with g = c%4. Each core computes Q/K/V projections for its
heads over the full sequence, causal differential attention, and its
partial O-projection; the host sums the 4 partials per batch.

Differential attention trick: score = (q1.k1 - lam*q2.k2) * scale is a
single K=128 matmul with stacked [q1*scale; -lam*scale*q2] and [k1; k2]
head vectors (scales folded into the projection weights on the host).

Softmax: scores are computed transposed (keys on partitions, queries
free), exp'd without max subtraction (inputs are bounded; exp is exact
to 2ULP on ACT), and the denominator comes for free from a ones-column
appended to V in the P@V matmul. Causality is applied structurally
(upper blocks skipped, diagonal blocks zeroed post-exp), which the host
validates against the attention_mask input before dispatch.
"""
import math
from contextlib import ExitStack

import numpy as np
import ml_dtypes

S = 2048
H = 2048
NH = 16
HD = 64
NHC = 4          # heads per core
BF = ml_dtypes.bfloat16

_CACHED_NC = None


def _build_nc():
    import concourse.mybir as mybir
    import concourse.tile as tile
    from concourse import bacc

    bf16 = mybir.dt.bfloat16
    f32 = mybir.dt.float32
    AF = mybir.ActivationFunctionType

    nc = bacc.Bacc(None, target_bir_lowering=False)
    hT = nc.declare_dram_parameter("hT", [H, S], bf16, isOutput=False)
    wq = nc.declare_dram_parameter("wq", [H, NHC * 128], bf16, isOutput=False)
    wk = nc.declare_dram_parameter("wk", [H, NHC * 128], bf16, isOutput=False)
    wv = nc.declare_dram_parameter("wv", [H, NHC * 65], bf16, isOutput=False)
    wo = nc.declare_dram_parameter("wo", [128, 2, S], bf16, isOutput=False)
    out = nc.declare_dram_parameter("out", [S, H], f32, isOutput=True)

    KT = H // 128    # 16 contraction tiles for projections
    NQ = S // 512    # 4 query chunks
    NS = S // 128    # 16 seq tiles

    with tile.TileContext(nc) as tc:
        with ExitStack() as ctx:
            # ---- persistent SBUF ----
            sb = ctx.enter_context(tc.tile_pool(name="sb", bufs=1))
            qk_sb = ctx.enter_context(tc.tile_pool(name="qk", bufs=1))
            ht_all = sb.tile([128, KT, S], bf16)          # hidden^T
            wq_sb = sb.tile([128, KT, NHC * 128], bf16)
            wk_sb = sb.tile([128, KT, NHC * 128], bf16)
            wv_sb = sb.tile([128, KT, NHC * 65], bf16)
            wo_sb = sb.tile([128, 2, S], bf16)            # head-pair stacked Wo rows
            qT = qk_sb.tile([128, NHC, S], bf16)          # [q1*s; -lam*s*q2] per head
            kT = qk_sb.tile([128, NHC, S], bf16)          # [k1; k2] per head
            v4 = qk_sb.tile([128, NS, NHC * 65], bf16)    # V tiles + ones cols
            avt = qk_sb.tile([128, 2, S], bf16)           # attn_out^T, head pairs stacked

            # ---- input DMAs: k-ordered contiguous rows so projection
            # chains (k-outer) can start as soon as the first rows land ----
            for k in range(KT):
                nc.sync.dma_start(out=ht_all[:, k, :], in_=hT[k * 128:(k + 1) * 128, :])
                nc.sync.dma_start(out=wq_sb[:, k, :], in_=wq[k * 128:(k + 1) * 128, :])
                nc.sync.dma_start(out=wk_sb[:, k, :], in_=wk[k * 128:(k + 1) * 128, :])
                nc.sync.dma_start(out=wv_sb[:, k, :], in_=wv[k * 128:(k + 1) * 128, :])
            nc.sync.dma_start(out=wo_sb[:], in_=wo[:, :, :])

            # ---- phase 1a: q/k projections (k-outer, nj-inner) ----
            with tc.tile_pool(name="pjp", bufs=2, space="PSUM") as pjp:
                for h in range(NHC):
                    hs = slice(h * 128, (h + 1) * 128)
                    for w_sb, dst in ((wq_sb, qT), (wk_sb, kT)):
                        pp = pjp.tile([128, S], f32, tag="qkp")
                        for k in range(KT):
                            for nj in range(NQ):
                                nc.tensor.matmul(pp[:, nj * 512:(nj + 1) * 512],
                                                 lhsT=w_sb[:, k, hs],
                                                 rhs=ht_all[:, k, nj * 512:(nj + 1) * 512],
                                                 start=(k == 0), stop=(k == KT - 1))
                        for nj in range(NQ):
                            nc.vector.tensor_copy(dst[:, h, nj * 512:(nj + 1) * 512],
                                                  pp[:, nj * 512:(nj + 1) * 512])

            # ---- phase 1b: V projection (natural [S, 260] layout) ----
            with tc.tile_pool(name="vjp", bufs=3, space="PSUM") as vjp:
                for st in range(NS):
                    vp = vjp.tile([128, NHC * 65], f32, tag="vp")
                    for k in range(KT):
                        nc.tensor.matmul(vp[:], lhsT=ht_all[:, k, st * 128:(st + 1) * 128],
                                         rhs=wv_sb[:, k, :],
                                         start=(k == 0), stop=(k == KT - 1))
                    nc.scalar.copy(v4[:, st, :], vp[:])
                    for j in range(NHC):
                        nc.gpsimd.memset(v4[:, st, j * 65 + 64:j * 65 + 65], 1.0)

            # ---- phase 2: attention ----
            att_work = ctx.enter_context(tc.tile_pool(name="attw", bufs=3))
            nrm_work = ctx.enter_context(tc.tile_pool(name="nrmw", bufs=2))
            with tc.tile_pool(name="atp", bufs=2, space="PSUM") as atp:
                for nj in range(NQ):
                    qs = slice(nj * 512, (nj + 1) * 512)
                    nblk = 4 * nj + 4
                    avs = []
                    for h in range(NHC):
                        av = atp.tile([65, 512], f32, tag="av", bufs=3)
                        avs.append(av)
                        for kg in range(nblk // 2):
                            sc = atp.tile([128, 1024], f32, tag="sc")
                            for u in range(2):
                                ki = 2 * kg + u
                                nc.tensor.matmul(sc[:, u * 512:(u + 1) * 512],
                                                 lhsT=kT[:, h, ki * 128:(ki + 1) * 128],
                                                 rhs=qT[:, h, qs], start=True, stop=True)
                            pt = att_work.tile([128, 1024], bf16, tag="pt")
                            # columns below 128*u of a diagonal block are fully
                            # masked; skip them in exp and the P@V matmul.
                            g0 = 256 if kg == 2 * nj + 1 else 0
                            nc.scalar.activation(pt[:, g0:1024], sc[:, g0:1024], AF.Exp)
                            for u in range(2):
                                ki = 2 * kg + u
                                uu = ki - 4 * nj  # >=0 on diagonal blocks
                                if uu >= 0:
                                    b0 = u * 512 + uu * 128
                                    nc.gpsimd.affine_select(
                                        out=pt[:, b0:b0 + 128],
                                        in_=pt[:, b0:b0 + 128],
                                        compare_op=mybir.AluOpType.is_ge,
                                        fill=0.0,
                                        base=0,
                                        channel_multiplier=-1,
                                        pattern=[[1, 128]],
                                    )
                                    nc.tensor.matmul(av[:, uu * 128:512],
                                                     lhsT=v4[:, ki, h * 65:(h + 1) * 65],
                                                     rhs=pt[:, u * 512 + uu * 128:(u + 1) * 512],
                                                     start=(ki == 0), stop=(ki == nblk - 1))
                                else:
                                    nc.tensor.matmul(av[:],
                                                     lhsT=v4[:, ki, h * 65:(h + 1) * 65],
                                                     rhs=pt[:, u * 512:(u + 1) * 512],
                                                     start=(ki == 0), stop=(ki == nblk - 1))
                    # normalize after all heads of this chunk: row 64 of av
                    # is the softmax denominator; scale rows 0-63 by 1/denom
                    # (reciprocal as exp(-ln(x)) on ScalarE; both fns share
                    # the natural_log_exp_and_others table set)
                    for h in range(NHC):
                        av = avs[h]
                        rc = nrm_work.tile([65, 512], f32, tag="rc")
                        nc.scalar.activation(rc[64:65, :], av[64:65, :], AF.Ln)
                        nc.scalar.activation(rc[64:65, :], rc[64:65, :], AF.Exp, scale=-1.0)
                        rc0 = nrm_work.tile([1, 512], f32, tag="rc0")
                        nc.sync.dma_start(out=rc0[:], in_=rc[64:65, :])
                        bcs = nrm_work.tile([64, 512], f32, tag="bcs")
                        nc.gpsimd.partition_broadcast(bcs[:], rc0[:])
                        pair, odd = divmod(h, 2)
                        if odd:
                            om = nrm_work.tile([64, 512], bf16, tag="om")
                            nc.vector.tensor_mul(om[:], av[0:64, :], bcs[:])
                            nc.sync.dma_start(out=avt[64:128, pair, qs], in_=om[:])
                        else:
                            nc.vector.tensor_mul(avt[0:64, pair, qs], av[0:64, :], bcs[:])

            # ---- phase 3: output projection (head pairs, K=128) ----
            oout_sb = ctx.enter_context(tc.tile_pool(name="oout", bufs=4))
            with tc.tile_pool(name="opp", bufs=4, space="PSUM") as opp:
                for qi in range(NS):
                    for nch in range(NQ):
                        op = opp.tile([128, 512], f32, tag="op")
                        for p in range(2):
                            nc.tensor.matmul(op[:],
                                             lhsT=avt[:, p, qi * 128:(qi + 1) * 128],
                                             rhs=wo_sb[:, p, nch * 512:(nch + 1) * 512],
                                             start=(p == 0), stop=(p == 1))
                        ot = oout_sb.tile([128, 512], f32, tag="ot")
                        nc.vector.tensor_copy(ot[:], op[:])
                        nc.sync.dma_start(
                            out=out[qi * 128:(qi + 1) * 128, nch * 512:(nch + 1) * 512],
                            in_=ot[:])
    return nc


def _get_nc():
    global _CACHED_NC
    if _CACHED_NC is None:
        nc = _build_nc()
        if not nc.is_finalized():
            nc.finalize()
        _CACHED_NC = nc
    return _CACHED_NC


def _prep_in_maps(hidden_states, Wq, Wk, Wv, Wo, lambda_param):
    lam = math.tanh(math.log1p(math.exp(float(lambda_param))))
    scale = HD ** -0.5
    in_maps = []
    hTb = [np.ascontiguousarray(hidden_states[b].T).astype(BF) for b in range(2)]
    for core in range(8):
        b, g = divmod(core, 4)
        heads = range(NHC * g, NHC * g + NHC)
        wq_cols, wk_cols = [], []
        for h in heads:
            wq_cols.append(Wq[:, h * 64:(h + 1) * 64] * scale)
            wq_cols.append(Wq[:, (NH + h) * 64:(NH + h + 1) * 64] * (-lam * scale))
            wk_cols.append(Wk[:, h * 64:(h + 1) * 64])
            wk_cols.append(Wk[:, (NH + h) * 64:(NH + h + 1) * 64])
        wv_pad = np.zeros((H, NHC * 65), dtype=np.float32)
        for j, h in enumerate(heads):
            wv_pad[:, j * 65:j * 65 + 64] = Wv[:, h * 64:(h + 1) * 64]
        heads = list(heads)
        wo_sel = np.zeros((128, 2, S), dtype=np.float32)  # head-pair stacked rows
        for p in range(2):
            h0, h1 = heads[2 * p], heads[2 * p + 1]
            wo_sel[0:64, p] = Wo[h0 * 64:(h0 + 1) * 64, :]
            wo_sel[64:128, p] = Wo[h1 * 64:(h1 + 1) * 64, :]
        in_maps.append({
            "hT": hTb[b],
            "wq": np.concatenate(wq_cols, axis=1).astype(BF),
            "wk": np.concatenate(wk_cols, axis=1).astype(BF),
            "wv": wv_pad.astype(BF),
            "wo": np.ascontiguousarray(wo_sel).astype(BF),
        })
    return in_maps


def _mask_is_causal(attention_mask):
    m = np.asarray(attention_mask)
    if m.shape != (2, 1, S, S):
        return False
    neg = np.float32(np.finfo(np.float32).min)
    # sampled structural check + full verification
    tri = np.tril(np.ones((S, S), dtype=bool))
    expect = np.where(tri, np.float32(0.0), neg)
    return all(np.array_equal(m[b, 0], expect) for b in range(m.shape[0]))


def _fallback(hidden_states, attention_mask, Wq, Wk, Wv, Wo, lambda_param):
    hs = hidden_states.astype(np.float32)
    lam = math.tanh(math.log1p(math.exp(float(lambda_param))))
    scaling = HD ** -0.5
    B = hs.shape[0]
    out = np.empty((B, S, H), dtype=np.float32)
    for b in range(B):
        q_all = (hs[b] @ Wq).reshape(S, 2 * NH, HD).transpose(1, 0, 2)
        k_all = (hs[b] @ Wk).reshape(S, 2 * NH, HD).transpose(1, 0, 2)
        v = (hs[b] @ Wv).reshape(S, NH, HD).transpose(1, 0, 2)
        acc = np.zeros((S, H), dtype=np.float32)
        for h in range(NH):
            s1 = q_all[h] @ k_all[h].T
            s2 = q_all[NH + h] @ k_all[NH + h].T
            sc = (s1 - lam * s2) * scaling + attention_mask[b, 0]
            sc -= sc.max(axis=-1, keepdims=True)
            p = np.exp(sc)
            p /= p.sum(axis=-1, keepdims=True)
            acc += (p @ v[h]) @ Wo[h * 64:(h + 1) * 64]
        out[b] = acc
    return out


def _run(inputs, trace=False):
    from concourse.bass_utils import run_bass_kernel_spmd

    hidden_states = np.asarray(inputs["hidden_states"], dtype=np.float32)
    attention_mask = np.asarray(inputs["attention_mask"], dtype=np.float32)
    Wq = np.asarray(inputs["Wq"], dtype=np.float32)
    Wk = np.asarray(inputs["Wk"], dtype=np.float32)
    Wv = np.asarray(inputs["Wv"], dtype=np.float32)
    Wo = np.asarray(inputs["Wo"], dtype=np.float32)
    lam_p = inputs["lambda_param"]

    if not _mask_is_causal(attention_mask):
        return _fallback(hidden_states, attention_mask, Wq, Wk, Wv, Wo, lam_p), None

    in_maps = _prep_in_maps(hidden_states, Wq, Wk, Wv, Wo, lam_p)
    nc = _get_nc()
    res = run_bass_kernel_spmd(nc, in_maps, list(range(8)), trace=trace)
    out = np.empty((2, S, H), dtype=np.float32)
    for b in range(2):
        acc = res.results[4 * b]["out"].astype(np.float32)
        for g in range(1, 4):
            acc = acc + res.results[4 * b + g]["out"]
        out[b] = acc
    return out, res


def kernel(**inputs):
    out, _ = _run(inputs, trace=False)
    return out


# revision 13
# speedup vs baseline: 1.0688x; 1.0688x over previous
"""Differential attention kernel for 8 Trainium2 NeuronCores.

Sharding: batch x head-group. Core c handles batch b = c//4, heads
[4g, 4g+4) with g = c%4. Each core computes Q/K/V projections for its
heads over the full sequence, causal differential attention, and its
partial O-projection; the host sums the 4 partials per batch.

Differential attention trick: score = (q1.k1 - lam*q2.k2) * scale is a
single K=128 matmul with stacked [q1*scale; -lam*scale*q2] and [k1; k2]
head vectors (scales folded into the projection weights on the host).

Softmax: scores are computed transposed (keys on partitions, queries
free), exp'd without max subtraction (inputs are bounded; exp is exact
to 2ULP on ACT), and the denominator comes for free from a ones-column
appended to V in the P@V matmul. Causality is applied structurally
(upper blocks skipped, diagonal blocks zeroed post-exp), which the host
validates against the attention_mask input before dispatch.
"""
import math
from contextlib import ExitStack

import numpy as np
import ml_dtypes

S = 2048
H = 2048
NH = 16
HD = 64
NHC = 4          # heads per core
BF = ml_dtypes.bfloat16

_CACHED_NC = None


def _build_nc():
    import concourse.mybir as mybir
    import concourse.tile as tile
    from concourse import bacc

    bf16 = mybir.dt.bfloat16
    f32 = mybir.dt.float32
    AF = mybir.ActivationFunctionType

    nc = bacc.Bacc(None, target_bir_lowering=False)
    hT = nc.declare_dram_parameter("hT", [H, S], bf16, isOutput=False)
    wq = nc.declare_dram_parameter("wq", [H, NHC * 128], bf16, isOutput=False)
    wk = nc.declare_dram_parameter("wk", [H, NHC * 128], bf16, isOutput=False)
    wv = nc.declare_dram_parameter("wv", [H, NHC * 65], bf16, isOutput=False)
    wo = nc.declare_dram_parameter("wo", [128, 2, S], bf16, isOutput=False)
    out = nc.declare_dram_parameter("out", [S, H], f32, isOutput=True)

    KT = H // 128    # 16 contraction tiles for projections
    NQ = S // 512    # 4 query chunks
    NS = S // 128    # 16 seq tiles

    with tile.TileContext(nc) as tc:
        with ExitStack() as ctx:
            # ---- persistent SBUF ----
            sb = ctx.enter_context(tc.tile_pool(name="sb", bufs=1))
            qk_sb = ctx.enter_context(tc.tile_pool(name="qk", bufs=1))
            ht_all = sb.tile([128, KT, S], bf16)          # hidden^T
            wq_sb = sb.tile([128, KT, NHC * 128], bf16)
            wk_sb = sb.tile([128, KT, NHC * 128], bf16)
            wv_sb = sb.tile([128, KT, NHC * 65], bf16)
            wo_sb = sb.tile([128, 2, S], bf16)            # head-pair stacked Wo rows
            qT = qk_sb.tile([128, NHC, S], bf16)          # [q1*s; -lam*s*q2] per head
            kT = qk_sb.tile([128, NHC, S], bf16)          # [k1; k2] per head
            v4 = qk_sb.tile([128, NS, NHC * 65], bf16)    # V tiles + ones cols
            avt = qk_sb.tile([128, 2, S], bf16)           # attn_out^T, head pairs stacked

            # ---- input DMAs: k-ordered contiguous rows so projection
            # chains (k-outer) can start as soon as the first rows land ----
            for k in range(KT):
                nc.sync.dma_start(out=ht_all[:, k, :], in_=hT[k * 128:(k + 1) * 128, :])
                nc.sync.dma_start(out=wq_sb[:, k, :], in_=wq[k * 128:(k + 1) * 128, :])
                nc.sync.dma_start(out=wk_sb[:, k, :], in_=wk[k * 128:(k + 1) * 128, :])
                nc.sync.dma_start(out=wv_sb[:, k, :], in_=wv[k * 128:(k + 1) * 128, :])
            nc.sync.dma_start(out=wo_sb[:], in_=wo[:, :, :])

            # ---- phase 1a: q/k projections (k-outer, nj-inner) ----
            with tc.tile_pool(name="pjp", bufs=2, space="PSUM") as pjp:
                for h in range(NHC):
                    hs = slice(h * 128, (h + 1) * 128)
                    for w_sb, dst in ((wq_sb, qT), (wk_sb, kT)):
                        pp = pjp.tile([128, S], f32, tag="qkp")
                        for k in range(KT):
                            for nj in range(NQ):
                                nc.tensor.matmul(pp[:, nj * 512:(nj + 1) * 512],
                                                 lhsT=w_sb[:, k, hs],
                                                 rhs=ht_all[:, k, nj * 512:(nj + 1) * 512],
                                                 start=(k == 0), stop=(k == KT - 1))
                        for nj in range(NQ):
                            nc.vector.tensor_copy(dst[:, h, nj * 512:(nj + 1) * 512],
                                                  pp[:, nj * 512:(nj + 1) * 512])

            # ---- phase 1b: V projection (natural [S, 260] layout) ----
            with tc.tile_pool(name="vjp", bufs=3, space="PSUM") as vjp:
                for st in range(NS):
                    vp = vjp.tile([128, NHC * 65], f32, tag="vp")
                    for k in range(KT):
                        nc.tensor.matmul(vp[:], lhsT=ht_all[:, k, st * 128:(st + 1) * 128],
                                         rhs=wv_sb[:, k, :],
                                         start=(k == 0), stop=(k == KT - 1))
                    nc.scalar.copy(v4[:, st, :], vp[:])
                    for j in range(NHC):
                        nc.gpsimd.memset(v4[:, st, j * 65 + 64:j * 65 + 65], 1.0)

            # ---- phase 2: attention ----
            att_work = ctx.enter_context(tc.tile_pool(name="attw", bufs=3))
            nrm_work = ctx.enter_context(tc.tile_pool(name="nrmw", bufs=2))
            with tc.tile_pool(name="atp", bufs=2, space="PSUM") as atp:
                for nj in range(NQ):
                    qs = slice(nj * 512, (nj + 1) * 512)
                    nblk = 4 * nj + 4
                    avs = []
                    for h in range(NHC):
                        av = atp.tile([65, 512], f32, tag="av", bufs=4)
                        avs.append(av)
                        for kg in range(nblk // 2):
                            sc = atp.tile([128, 1024], f32, tag="sc")
                            for u in range(2):
                                ki = 2 * kg + u
                                nc.tensor.matmul(sc[:, u * 512:(u + 1) * 512],
                                                 lhsT=kT[:, h, ki * 128:(ki + 1) * 128],
                                                 rhs=qT[:, h, qs], start=True, stop=True)
                            pt = att_work.tile([128, 1024], bf16, tag="pt")
                            # columns below 128*u of a diagonal block are fully
                            # masked; skip them in exp and the P@V matmul.
                            g0 = 256 if kg == 2 * nj + 1 else 0
                            nc.scalar.activation(pt[:, g0:1024], sc[:, g0:1024], AF.Exp)
                            for u in range(2):
                                ki = 2 * kg + u
                                uu = ki - 4 * nj  # >=0 on diagonal blocks
                                if uu >= 0:
                                    b0 = u * 512 + uu * 128
                                    nc.gpsimd.affine_select(
                                        out=pt[:, b0:b0 + 128],
                                        in_=pt[:, b0:b0 + 128],
                                        compare_op=mybir.AluOpType.is_ge,
                                        fill=0.0,
                                        base=0,
                                        channel_multiplier=-1,
                                        pattern=[[1, 128]],
                                    )
                                    nc.tensor.matmul(av[:, uu * 128:512],
                                                     lhsT=v4[:, ki, h * 65:(h + 1) * 65],
                                                     rhs=pt[:, u * 512 + uu * 128:(u + 1) * 512],
                                                     start=(ki == 0), stop=(ki == nblk - 1))
                                else:
                                    nc.tensor.matmul(av[:],
                                                     lhsT=v4[:, ki, h * 65:(h + 1) * 65],
                                                     rhs=pt[:, u * 512:(u + 1) * 512],
                                                     start=(ki == 0), stop=(ki == nblk - 1))
                    # normalize after all heads of this chunk: row 64 of av
                    # is the softmax denominator; scale rows 0-63 by 1/denom.
                    # Reciprocal runs on a [128, 16] DMA-folded layout so all
                    # 128 DVE lanes work instead of one.
                    dfold = nrm_work.tile([128, 16], f32, tag="dfold")
                    for h in range(NHC):
                        den = nrm_work.tile([65, 512], f32, tag="den", bufs=4)
                        nc.scalar.copy(den[64:65, :], avs[h][64:65, :])
                        nc.sync.dma_start(out=dfold[:, 4 * h:4 * h + 4], in_=den[64:65, :])
                    nc.vector.reciprocal(dfold[:], dfold[:])
                    for h in range(NHC):
                        av = avs[h]
                        rc0 = nrm_work.tile([1, 512], f32, tag="rc0")
                        nc.sync.dma_start(out=rc0[:], in_=dfold[:, 4 * h:4 * h + 4])
                        bcs = nrm_work.tile([64, 512], f32, tag="bcs")
                        nc.gpsimd.partition_broadcast(bcs[:], rc0[:])
                        pair, odd = divmod(h, 2)
                        if odd:
                            om = nrm_work.tile([64, 512], bf16, tag="om")
                            nc.vector.tensor_mul(om[:], av[0:64, :], bcs[:])
                            nc.sync.dma_start(out=avt[64:128, pair, qs], in_=om[:])
                        else:
                            nc.vector.tensor_mul(avt[0:64, pair, qs], av[0:64, :], bcs[:])

            # ---- phase 3: output projection (head pairs, K=128) ----
            oout_sb = ctx.enter_context(tc.tile_pool(name="oout", bufs=4))
            with tc.tile_pool(name="opp", bufs=4, space="PSUM") as opp:
                for qi in range(NS):
                    for nch in range(NQ):
                        op = opp.tile([128, 512], f32, tag="op")
                        for p in range(2):
                            nc.tensor.matmul(op[:],
                                             lhsT=avt[:, p, qi * 128:(qi + 1) * 128],
                                             rhs=wo_sb[:, p, nch * 512:(nch + 1) * 512],
                                             start=(p == 0), stop=(p == 1))
                        ot = oout_sb.tile([128, 512], f32, tag="ot")
                        nc.vector.tensor_copy(ot[:], op[:])
                        nc.sync.dma_start(
                            out=out[qi * 128:(qi + 1) * 128, nch * 512:(nch + 1) * 512],
                            in_=ot[:])
    return nc


def _get_nc():
    global _CACHED_NC
    if _CACHED_NC is None:
        nc = _build_nc()
        if not nc.is_finalized():
            nc.finalize()
        _CACHED_NC = nc
    return _CACHED_NC


def _prep_in_maps(hidden_states, Wq, Wk, Wv, Wo, lambda_param):
    lam = math.tanh(math.log1p(math.exp(float(lambda_param))))
    scale = HD ** -0.5
    in_maps = []
    hTb = [np.ascontiguousarray(hidden_states[b].T).astype(BF) for b in range(2)]
    for core in range(8):
        b, g = divmod(core, 4)
        heads = range(NHC * g, NHC * g + NHC)
        wq_cols, wk_cols = [], []
        for h in heads:
            wq_cols.append(Wq[:, h * 64:(h + 1) * 64] * scale)
            wq_cols.append(Wq[:, (NH + h) * 64:(NH + h + 1) * 64] * (-lam * scale))
            wk_cols.append(Wk[:, h * 64:(h + 1) * 64])
            wk_cols.append(Wk[:, (NH + h) * 64:(NH + h + 1) * 64])
        wv_pad = np.zeros((H, NHC * 65), dtype=np.float32)
        for j, h in enumerate(heads):
            wv_pad[:, j * 65:j * 65 + 64] = Wv[:, h * 64:(h + 1) * 64]
        heads = list(heads)
        wo_sel = np.zeros((128, 2, S), dtype=np.float32)  # head-pair stacked rows
        for p in range(2):
            h0, h1 = heads[2 * p], heads[2 * p + 1]
            wo_sel[0:64, p] = Wo[h0 * 64:(h0 + 1) * 64, :]
            wo_sel[64:128, p] = Wo[h1 * 64:(h1 + 1) * 64, :]
        in_maps.append({
            "hT": hTb[b],
            "wq": np.concatenate(wq_cols, axis=1).astype(BF),
            "wk": np.concatenate(wk_cols, axis=1).astype(BF),
            "wv": wv_pad.astype(BF),
            "wo": np.ascontiguousarray(wo_sel).astype(BF),
        })
    return in_maps


def _mask_is_causal(attention_mask):
    m = np.asarray(attention_mask)
    if m.shape != (2, 1, S, S):
        return False
    neg = np.float32(np.finfo(np.float32).min)
    # sampled structural check + full verification
    tri = np.tril(np.ones((S, S), dtype=bool))
    expect = np.where(tri, np.float32(0.0), neg)
    return all(np.array_equal(m[b, 0], expect) for b in range(m.shape[0]))


def _fallback(hidden_states, attention_mask, Wq, Wk, Wv, Wo, lambda_param):
    hs = hidden_states.astype(np.float32)
    lam = math.tanh(math.log1p(math.exp(float(lambda_param))))
    scaling = HD ** -0.5
    B = hs.shape[0]
    out = np.empty((B, S, H), dtype=np.float32)
    for b in range(B):
        q_all = (hs[b] @ Wq).reshape(S, 2 * NH, HD).transpose(1, 0, 2)
        k_all = (hs[b] @ Wk).reshape(S, 2 * NH, HD).transpose(1, 0, 2)
        v = (hs[b] @ Wv).reshape(S, NH, HD).transpose(1, 0, 2)
        acc = np.zeros((S, H), dtype=np.float32)
        for h in range(NH):
            s1 = q_all[h] @ k_all[h].T
            s2 = q_all[NH + h] @ k_all[NH + h].T
            sc = (s1 - lam * s2) * scaling + attention_mask[b, 0]
            sc -= sc.max(axis=-1, keepdims=True)
            p = np.exp(sc)
            p /= p.sum(axis=-1, keepdims=True)
            acc += (p @ v[h]) @ Wo[h * 64:(h + 1) * 64]
        out[b] = acc
    return out


def _run(inputs, trace=False):
    from concourse.bass_utils import run_bass_kernel_spmd

    hidden_states = np.asarray(inputs["hidden_states"], dtype=np.float32)
    attention_mask = np.asarray(inputs["attention_mask"], dtype=np.float32)
    Wq = np.asarray(inputs["Wq"], dtype=np.float32)
    Wk = np.asarray(inputs["Wk"], dtype=np.float32)
    Wv = np.asarray(inputs["Wv"], dtype=np.float32)
    Wo = np.asarray(inputs["Wo"], dtype=np.float32)
    lam_p = inputs["lambda_param"]

    if not _mask_is_causal(attention_mask):
        return _fallback(hidden_states, attention_mask, Wq, Wk, Wv, Wo, lam_p), None

    in_maps = _prep_in_maps(hidden_states, Wq, Wk, Wv, Wo, lam_p)
    nc = _get_nc()
    res = run_bass_kernel_spmd(nc, in_maps, list(range(8)), trace=trace)
    out = np.empty((2, S, H), dtype=np.float32)
    for b in range(2):
        acc = res.results[4 * b]["out"].astype(np.float32)
        for g in range(1, 4):
            acc = acc + res.results[4 * b + g]["out"]
        out[b] = acc
    return out, res


def kernel(**inputs):
    out, _ = _run(inputs, trace=False)
    return out


# revision 15
# speedup vs baseline: 1.1081x; 1.0367x over previous
"""Differential attention kernel for 8 Trainium2 NeuronCores.

Sharding: batch x head-group. Core c handles batch b = c//4, heads
[4g, 4g+4) with g = c%4. Each core computes Q/K/V projections for its
heads over the full sequence, causal differential attention, and its
partial O-projection; the host sums the 4 partials per batch.

Differential attention trick: score = (q1.k1 - lam*q2.k2) * scale is a
single K=128 matmul with stacked [q1*scale; -lam*scale*q2] and [k1; k2]
head vectors (scales folded into the projection weights on the host).

Softmax: scores are computed transposed (keys on partitions, queries
free), exp'd without max subtraction (inputs are bounded; exp is exact
to 2ULP on ACT), and the denominator comes for free from a ones-column
appended to V in the P@V matmul. Causality is applied structurally
(upper blocks skipped, diagonal blocks zeroed post-exp), which the host
validates against the attention_mask input before dispatch.
"""
import math
from contextlib import ExitStack

import numpy as np
import ml_dtypes

S = 2048
H = 2048
NH = 16
HD = 64
NHC = 4          # heads per core
BF = ml_dtypes.bfloat16

_CACHED_NC = None


def _build_nc():
    import concourse.mybir as mybir
    import concourse.tile as tile
    from concourse import bacc

    bf16 = mybir.dt.bfloat16
    f32 = mybir.dt.float32
    AF = mybir.ActivationFunctionType

    nc = bacc.Bacc(None, target_bir_lowering=False)
    hT = nc.declare_dram_parameter("hT", [H, S], bf16, isOutput=False)
    wq = nc.declare_dram_parameter("wq", [H, NHC * 128], bf16, isOutput=False)
    wk = nc.declare_dram_parameter("wk", [H, NHC * 128], bf16, isOutput=False)
    wv = nc.declare_dram_parameter("wv", [H, NHC * 65], bf16, isOutput=False)
    wo = nc.declare_dram_parameter("wo", [128, 2, S], bf16, isOutput=False)
    out = nc.declare_dram_parameter("out", [S, H], f32, isOutput=True)

    KT = H // 128    # 16 contraction tiles for projections
    NQ = S // 512    # 4 query chunks
    NS = S // 128    # 16 seq tiles

    with tile.TileContext(nc) as tc:
        with ExitStack() as ctx:
            # ---- persistent SBUF ----
            sb = ctx.enter_context(tc.tile_pool(name="sb", bufs=1))
            qk_sb = ctx.enter_context(tc.tile_pool(name="qk", bufs=1))
            ht_all = sb.tile([128, KT, S], bf16)          # hidden^T
            wq_sb = sb.tile([128, KT, NHC * 128], bf16)
            wk_sb = sb.tile([128, KT, NHC * 128], bf16)
            wv_sb = sb.tile([128, KT, NHC * 65], bf16)
            wo_sb = sb.tile([128, 2, S], bf16)            # head-pair stacked Wo rows
            qT = qk_sb.tile([128, NHC, S], bf16)          # [q1*s; -lam*s*q2] per head
            kT = qk_sb.tile([128, NHC, S], bf16)          # [k1; k2] per head
            v4 = qk_sb.tile([128, NS, NHC * 65], bf16)    # V tiles + ones cols
            avt = qk_sb.tile([128, 2, S], bf16)           # attn_out^T, head pairs stacked

            # ---- input DMAs: k-ordered contiguous rows so projection
            # chains (k-outer) can start as soon as the first rows land ----
            for k in range(KT):
                nc.sync.dma_start(out=ht_all[:, k, :], in_=hT[k * 128:(k + 1) * 128, :])
                nc.sync.dma_start(out=wq_sb[:, k, :], in_=wq[k * 128:(k + 1) * 128, :])
                nc.sync.dma_start(out=wk_sb[:, k, :], in_=wk[k * 128:(k + 1) * 128, :])
                nc.sync.dma_start(out=wv_sb[:, k, :], in_=wv[k * 128:(k + 1) * 128, :])
            nc.sync.dma_start(out=wo_sb[:], in_=wo[:, :, :])

            # ---- phase 1a: q/k projections (k-outer, nj-inner) ----
            with tc.tile_pool(name="pjp", bufs=2, space="PSUM") as pjp:
                for h in range(NHC):
                    hs = slice(h * 128, (h + 1) * 128)
                    for w_sb, dst in ((wq_sb, qT), (wk_sb, kT)):
                        pp = pjp.tile([128, S], f32, tag="qkp")
                        for k in range(KT):
                            for nj in range(NQ):
                                nc.tensor.matmul(pp[:, nj * 512:(nj + 1) * 512],
                                                 lhsT=w_sb[:, k, hs],
                                                 rhs=ht_all[:, k, nj * 512:(nj + 1) * 512],
                                                 start=(k == 0), stop=(k == KT - 1))
                        for nj in range(NQ):
                            nc.vector.tensor_copy(dst[:, h, nj * 512:(nj + 1) * 512],
                                                  pp[:, nj * 512:(nj + 1) * 512])

            # ---- phase 1b: V projection (natural [S, 260] layout) ----
            with tc.tile_pool(name="vjp", bufs=3, space="PSUM") as vjp:
                for st in range(NS):
                    vp = vjp.tile([128, NHC * 65], f32, tag="vp")
                    for k in range(KT):
                        nc.tensor.matmul(vp[:], lhsT=ht_all[:, k, st * 128:(st + 1) * 128],
                                         rhs=wv_sb[:, k, :],
                                         start=(k == 0), stop=(k == KT - 1))
                    if st % 2 == 0:
                        nc.scalar.copy(v4[:, st, :], vp[:])
                    else:
                        nc.vector.tensor_copy(v4[:, st, :], vp[:])
                    for j in range(NHC):
                        nc.gpsimd.memset(v4[:, st, j * 65 + 64:j * 65 + 65], 1.0)

            # ---- phase 2: attention ----
            att_work = ctx.enter_context(tc.tile_pool(name="attw", bufs=3))
            nrm_work = ctx.enter_context(tc.tile_pool(name="nrmw", bufs=2))
            with tc.tile_pool(name="atp", bufs=2, space="PSUM") as atp:
                for nj in range(NQ):
                    qs = slice(nj * 512, (nj + 1) * 512)
                    nblk = 4 * nj + 4
                    avs = []
                    for h in range(NHC):
                        av = atp.tile([65, 512], f32, tag="av", bufs=4)
                        avs.append(av)
                        for kg in range(nblk // 2):
                            sc = atp.tile([128, 1024], f32, tag="sc")
                            for u in range(2):
                                ki = 2 * kg + u
                                nc.tensor.matmul(sc[:, u * 512:(u + 1) * 512],
                                                 lhsT=kT[:, h, ki * 128:(ki + 1) * 128],
                                                 rhs=qT[:, h, qs], start=True, stop=True)
                            pt = att_work.tile([128, 1024], bf16, tag="pt")
                            # columns below 128*u of a diagonal block are fully
                            # masked; skip them in exp and the P@V matmul.
                            g0 = 256 if kg == 2 * nj + 1 else 0
                            nc.scalar.activation(pt[:, g0:1024], sc[:, g0:1024], AF.Exp)
                            for u in range(2):
                                ki = 2 * kg + u
                                uu = ki - 4 * nj  # >=0 on diagonal blocks
                                if uu >= 0:
                                    b0 = u * 512 + uu * 128
                                    nc.gpsimd.affine_select(
                                        out=pt[:, b0:b0 + 128],
                                        in_=pt[:, b0:b0 + 128],
                                        compare_op=mybir.AluOpType.is_ge,
                                        fill=0.0,
                                        base=0,
                                        channel_multiplier=-1,
                                        pattern=[[1, 128]],
                                    )
                                    nc.tensor.matmul(av[:, uu * 128:512],
                                                     lhsT=v4[:, ki, h * 65:(h + 1) * 65],
                                                     rhs=pt[:, u * 512 + uu * 128:(u + 1) * 512],
                                                     start=(ki == 0), stop=(ki == nblk - 1))
                                else:
                                    nc.tensor.matmul(av[:],
                                                     lhsT=v4[:, ki, h * 65:(h + 1) * 65],
                                                     rhs=pt[:, u * 512:(u + 1) * 512],
                                                     start=(ki == 0), stop=(ki == nblk - 1))
                    # normalize after all heads of this chunk: row 64 of av
                    # is the softmax denominator; scale rows 0-63 by 1/denom.
                    # Reciprocal runs on a [128, 16] DMA-folded layout so all
                    # 128 DVE lanes work instead of one.
                    dfold = nrm_work.tile([128, 16], f32, tag="dfold")
                    araws = []
                    for h in range(NHC):
                        den = nrm_work.tile([65, 512], f32, tag="den", bufs=4)
                        nc.scalar.copy(den[64:65, :], avs[h][64:65, :])
                        nc.sync.dma_start(out=dfold[:, 4 * h:4 * h + 4], in_=den[64:65, :])
                        # evacuate the numerator too so the PSUM bank frees
                        # without waiting for the normalization chain
                        araw = nrm_work.tile([64, 512], bf16, tag="araw", bufs=4)
                        nc.scalar.copy(araw[:], avs[h][0:64, :])
                        araws.append(araw)
                    nc.vector.reciprocal(dfold[:], dfold[:])
                    for h in range(NHC):
                        rc0 = nrm_work.tile([1, 512], f32, tag="rc0")
                        nc.sync.dma_start(out=rc0[:], in_=dfold[:, 4 * h:4 * h + 4])
                        bcs = nrm_work.tile([64, 512], f32, tag="bcs")
                        nc.gpsimd.partition_broadcast(bcs[:], rc0[:])
                        pair, odd = divmod(h, 2)
                        if odd:
                            om = nrm_work.tile([64, 512], bf16, tag="om")
                            nc.vector.tensor_mul(om[:], araws[h][:], bcs[:])
                            nc.sync.dma_start(out=avt[64:128, pair, qs], in_=om[:])
                        else:
                            nc.vector.tensor_mul(avt[0:64, pair, qs], araws[h][:], bcs[:])

            # ---- phase 3: output projection (head pairs, K=128) ----
            oout_sb = ctx.enter_context(tc.tile_pool(name="oout", bufs=4))
            with tc.tile_pool(name="opp", bufs=4, space="PSUM") as opp:
                for qi in range(NS):
                    for nch in range(NQ):
                        op = opp.tile([128, 512], f32, tag="op")
                        for p in range(2):
                            nc.tensor.matmul(op[:],
                                             lhsT=avt[:, p, qi * 128:(qi + 1) * 128],
                                             rhs=wo_sb[:, p, nch * 512:(nch + 1) * 512],
                                             start=(p == 0), stop=(p == 1))
                        ot = oout_sb.tile([128, 512], f32, tag="ot")
                        nc.vector.tensor_copy(ot[:], op[:])
                        nc.sync.dma_start(
                            out=out[qi * 128:(qi + 1) * 128, nch * 512:(nch + 1) * 512],
                            in_=ot[:])
    return nc


def _get_nc():
    global _CACHED_NC
    if _CACHED_NC is None:
        nc = _build_nc()
        if not nc.is_finalized():
            nc.finalize()
        _CACHED_NC = nc
    return _CACHED_NC


def _prep_in_maps(hidden_states, Wq, Wk, Wv, Wo, lambda_param):
    lam = math.tanh(math.log1p(math.exp(float(lambda_param))))
    scale = HD ** -0.5
    in_maps = []
    hTb = [np.ascontiguousarray(hidden_states[b].T).astype(BF) for b in range(2)]
    for core in range(8):
        b, g = divmod(core, 4)
        heads = range(NHC * g, NHC * g + NHC)
        wq_cols, wk_cols = [], []
        for h in heads:
            wq_cols.append(Wq[:, h * 64:(h + 1) * 64] * scale)
            wq_cols.append(Wq[:, (NH + h) * 64:(NH + h + 1) * 64] * (-lam * scale))
            wk_cols.append(Wk[:, h * 64:(h + 1) * 64])
            wk_cols.append(Wk[:, (NH + h) * 64:(NH + h + 1) * 64])
        wv_pad = np.zeros((H, NHC * 65), dtype=np.float32)
        for j, h in enumerate(heads):
            wv_pad[:, j * 65:j * 65 + 64] = Wv[:, h * 64:(h + 1) * 64]
        heads = list(heads)
        wo_sel = np.zeros((128, 2, S), dtype=np.float32)  # head-pair stacked rows
        for p in range(2):
            h0, h1 = heads[2 * p], heads[2 * p + 1]
            wo_sel[0:64, p] = Wo[h0 * 64:(h0 + 1) * 64, :]
            wo_sel[64:128, p] = Wo[h1 * 64:(h1 + 1) * 64, :]
        in_maps.append({
            "hT": hTb[b],
            "wq": np.concatenate(wq_cols, axis=1).astype(BF),
            "wk": np.concatenate(wk_cols, axis=1).astype(BF),
            "wv": wv_pad.astype(BF),
            "wo": np.ascontiguousarray(wo_sel).astype(BF),
        })
    return in_maps


def _mask_is_causal(attention_mask):
    m = np.asarray(attention_mask)
    if m.shape != (2, 1, S, S):
        return False
    neg = np.float32(np.finfo(np.float32).min)
    # sampled structural check + full verification
    tri = np.tril(np.ones((S, S), dtype=bool))
    expect = np.where(tri, np.float32(0.0), neg)
    return all(np.array_equal(m[b, 0], expect) for b in range(m.shape[0]))


def _fallback(hidden_states, attention_mask, Wq, Wk, Wv, Wo, lambda_param):
    hs = hidden_states.astype(np.float32)
    lam = math.tanh(math.log1p(math.exp(float(lambda_param))))
    scaling = HD ** -0.5
    B = hs.shape[0]
    out = np.empty((B, S, H), dtype=np.float32)
    for b in range(B):
        q_all = (hs[b] @ Wq).reshape(S, 2 * NH, HD).transpose(1, 0, 2)
        k_all = (hs[b] @ Wk).reshape(S, 2 * NH, HD).transpose(1, 0, 2)
        v = (hs[b] @ Wv).reshape(S, NH, HD).transpose(1, 0, 2)
        acc = np.zeros((S, H), dtype=np.float32)
        for h in range(NH):
            s1 = q_all[h] @ k_all[h].T
            s2 = q_all[NH + h] @ k_all[NH + h].T
            sc = (s1 - lam * s2) * scaling + attention_mask[b, 0]
            sc -= sc.max(axis=-1, keepdims=True)
            p = np.exp(sc)
            p /= p.sum(axis=-1, keepdims=True)
            acc += (p @ v[h]) @ Wo[h * 64:(h + 1) * 64]
        out[b] = acc
    return out


def _run(inputs, trace=False):
    from concourse.bass_utils import run_bass_kernel_spmd

    hidden_states = np.asarray(inputs["hidden_states"], dtype=np.float32)
    attention_mask = np.asarray(inputs["attention_mask"], dtype=np.float32)
    Wq = np.asarray(inputs["Wq"], dtype=np.float32)
    Wk = np.asarray(inputs["Wk"], dtype=np.float32)
    Wv = np.asarray(inputs["Wv"], dtype=np.float32)
    Wo = np.asarray(inputs["Wo"], dtype=np.float32)
    lam_p = inputs["lambda_param"]

    if not _mask_is_causal(attention_mask):
        return _fallback(hidden_states, attention_mask, Wq, Wk, Wv, Wo, lam_p), None

    in_maps = _prep_in_maps(hidden_states, Wq, Wk, Wv, Wo, lam_p)
    nc = _get_nc()
    res = run_bass_kernel_spmd(nc, in_maps, list(range(8)), trace=trace)
    out = np.empty((2, S, H), dtype=np.float32)
    for b in range(2):
        acc = res.results[4 * b]["out"].astype(np.float32)
        for g in range(1, 4):
            acc = acc + res.results[4 * b + g]["out"]
        out[b] = acc
    return out, res


def kernel(**inputs):
    out, _ = _run(inputs, trace=False)
    return out


# revision 17
# speedup vs baseline: 1.1535x; 1.0409x over previous
"""Differential attention kernel for 8 Trainium2 NeuronCores.

Sharding: batch x head-group. Core c handles batch b = c//4, heads
[4g, 4g+4) with g = c%4. Each core computes Q/K/V projections for its
heads over the full sequence, causal differential attention, and its
partial O-projection; the host sums the 4 partials per batch.

Differential attention trick: score = (q1.k1 - lam*q2.k2) * scale is a
single K=128 matmul with stacked [q1*scale; -lam*scale*q2] and [k1; k2]
head vectors (scales folded into the projection weights on the host).

Softmax: scores are computed transposed (keys on partitions, queries
free), exp'd without max subtraction (inputs are bounded; exp is exact
to 2ULP on ACT), and the denominator comes for free from a ones-column
appended to V in the P@V matmul. Causality is applied structurally
(upper blocks skipped, diagonal blocks zeroed post-exp), which the host
validates against the attention_mask input before dispatch.
"""
import math
from contextlib import ExitStack

import numpy as np
import ml_dtypes

S = 2048
H = 2048
NH = 16
HD = 64
NHC = 4          # heads per core
BF = ml_dtypes.bfloat16

_CACHED_NC = None


def _build_nc():
    import concourse.mybir as mybir
    import concourse.tile as tile
    from concourse import bacc

    bf16 = mybir.dt.bfloat16
    f32 = mybir.dt.float32
    AF = mybir.ActivationFunctionType

    nc = bacc.Bacc(None, target_bir_lowering=False)
    hT = nc.declare_dram_parameter("hT", [H, S], bf16, isOutput=False)
    wq = nc.declare_dram_parameter("wq", [H, NHC * 128], bf16, isOutput=False)
    wk = nc.declare_dram_parameter("wk", [H, NHC * 128], bf16, isOutput=False)
    wv = nc.declare_dram_parameter("wv", [H, NHC * 65], bf16, isOutput=False)
    wo = nc.declare_dram_parameter("wo", [128, 2, S], bf16, isOutput=False)
    out = nc.declare_dram_parameter("out", [S, H], f32, isOutput=True)

    KT = H // 128    # 16 contraction tiles for projections
    NQ = S // 512    # 4 query chunks
    NS = S // 128    # 16 seq tiles

    with tile.TileContext(nc) as tc:
        with ExitStack() as ctx:
            # ---- persistent SBUF ----
            sb = ctx.enter_context(tc.tile_pool(name="sb", bufs=1))
            qk_sb = ctx.enter_context(tc.tile_pool(name="qk", bufs=1))
            ht_all = sb.tile([128, KT, S], bf16)          # hidden^T
            wq_sb = sb.tile([128, KT, NHC * 128], bf16)
            wk_sb = sb.tile([128, KT, NHC * 128], bf16)
            wv_sb = sb.tile([128, KT, NHC * 65], bf16)
            wo_sb = sb.tile([128, 2, S], bf16)            # head-pair stacked Wo rows
            qT = qk_sb.tile([128, NHC, S], bf16)          # [q1*s; -lam*s*q2] per head
            kT = qk_sb.tile([128, NHC, S], bf16)          # [k1; k2] per head
            v4 = qk_sb.tile([128, NS, NHC * 65], bf16)    # V tiles + ones cols
            avt = qk_sb.tile([128, 2, S], bf16)           # attn_out^T, head pairs stacked

            # ---- input DMAs: k-ordered contiguous rows so projection
            # chains (k-outer) can start as soon as the first rows land ----
            for k in range(KT):
                nc.sync.dma_start(out=ht_all[:, k, :], in_=hT[k * 128:(k + 1) * 128, :])
                nc.sync.dma_start(out=wq_sb[:, k, :], in_=wq[k * 128:(k + 1) * 128, :])
                nc.sync.dma_start(out=wk_sb[:, k, :], in_=wk[k * 128:(k + 1) * 128, :])
                nc.sync.dma_start(out=wv_sb[:, k, :], in_=wv[k * 128:(k + 1) * 128, :])
            nc.sync.dma_start(out=wo_sb[:], in_=wo[:, :, :])

            # ---- phase 1a: q/k projections (k-outer, nj-inner) ----
            with tc.tile_pool(name="pjp", bufs=2, space="PSUM") as pjp:
                for h in range(NHC):
                    hs = slice(h * 128, (h + 1) * 128)
                    for w_sb, dst in ((wq_sb, qT), (wk_sb, kT)):
                        pp = pjp.tile([128, S], f32, tag="qkp")
                        for k in range(KT):
                            for nj in range(NQ):
                                nc.tensor.matmul(pp[:, nj * 512:(nj + 1) * 512],
                                                 lhsT=w_sb[:, k, hs],
                                                 rhs=ht_all[:, k, nj * 512:(nj + 1) * 512],
                                                 start=(k == 0), stop=(k == KT - 1))
                        for nj in range(NQ):
                            nc.vector.tensor_copy(dst[:, h, nj * 512:(nj + 1) * 512],
                                                  pp[:, nj * 512:(nj + 1) * 512])

            # ---- phase 1b: V projection (natural [S, 260] layout) ----
            with tc.tile_pool(name="vjp", bufs=3, space="PSUM") as vjp:
                for st in range(NS):
                    vp = vjp.tile([128, NHC * 65], f32, tag="vp")
                    for k in range(KT):
                        nc.tensor.matmul(vp[:], lhsT=ht_all[:, k, st * 128:(st + 1) * 128],
                                         rhs=wv_sb[:, k, :],
                                         start=(k == 0), stop=(k == KT - 1))
                    if st % 2 == 0:
                        nc.scalar.copy(v4[:, st, :], vp[:])
                    else:
                        nc.vector.tensor_copy(v4[:, st, :], vp[:])
                    for j in range(NHC):
                        nc.gpsimd.memset(v4[:, st, j * 65 + 64:j * 65 + 65], 1.0)

            # ---- phase 2: attention ----
            att_work = ctx.enter_context(tc.tile_pool(name="attw", bufs=3))
            nrm_work = ctx.enter_context(tc.tile_pool(name="nrmw", bufs=2))
            with tc.tile_pool(name="atp", bufs=2, space="PSUM") as atp:
                for nj in range(NQ):
                    qs = slice(nj * 512, (nj + 1) * 512)
                    nblk = 4 * nj + 4
                    avs = []
                    for hp in range(2):
                        pair_heads = (2 * hp, 2 * hp + 1)
                        pav = {}
                        for h in pair_heads:
                            pav[h] = atp.tile([65, 512], f32, tag="av", bufs=4, name=f"av{h}")
                            avs.append(pav[h])
                        # round-robin the two heads per key-block group so one
                        # head's score matmuls hide the other head's exp
                        for kg in range(nblk // 2):
                            scs, pts = {}, {}
                            g0 = 256 if kg == 2 * nj + 1 else 0
                            for h in pair_heads:
                                sc = atp.tile([128, 1024], f32, tag="sc", name=f"sc{h}")
                                scs[h] = sc
                                for u in range(2):
                                    ki = 2 * kg + u
                                    nc.tensor.matmul(sc[:, u * 512:(u + 1) * 512],
                                                     lhsT=kT[:, h, ki * 128:(ki + 1) * 128],
                                                     rhs=qT[:, h, qs], start=True, stop=True)
                            for h in pair_heads:
                                pt = att_work.tile([128, 1024], bf16, tag="pt", bufs=4, name=f"pt{h}")
                                pts[h] = pt
                                # columns below 128*uu of a diagonal block are
                                # fully masked; skip in exp and the P@V matmul
                                nc.scalar.activation(pt[:, g0:1024], scs[h][:, g0:1024], AF.Exp)
                            for h in pair_heads:
                                pt, av = pts[h], pav[h]
                                for u in range(2):
                                    ki = 2 * kg + u
                                    uu = ki - 4 * nj  # >=0 on diagonal blocks
                                    if uu >= 0:
                                        b0 = u * 512 + uu * 128
                                        nc.gpsimd.affine_select(
                                            out=pt[:, b0:b0 + 128],
                                            in_=pt[:, b0:b0 + 128],
                                            compare_op=mybir.AluOpType.is_ge,
                                            fill=0.0,
                                            base=0,
                                            channel_multiplier=-1,
                                            pattern=[[1, 128]],
                                        )
                                        nc.tensor.matmul(av[:, uu * 128:512],
                                                         lhsT=v4[:, ki, h * 65:(h + 1) * 65],
                                                         rhs=pt[:, u * 512 + uu * 128:(u + 1) * 512],
                                                         start=(ki == 0), stop=(ki == nblk - 1))
                                    else:
                                        nc.tensor.matmul(av[:],
                                                         lhsT=v4[:, ki, h * 65:(h + 1) * 65],
                                                         rhs=pt[:, u * 512:(u + 1) * 512],
                                                         start=(ki == 0), stop=(ki == nblk - 1))
                    # normalize after all heads of this chunk: row 64 of av
                    # is the softmax denominator; scale rows 0-63 by 1/denom.
                    # Reciprocal runs on a [128, 16] DMA-folded layout so all
                    # 128 DVE lanes work instead of one.
                    dfold = nrm_work.tile([128, 16], f32, tag="dfold")
                    araws = []
                    for h in range(NHC):
                        den = nrm_work.tile([65, 512], f32, tag="den", bufs=4)
                        nc.vector.tensor_copy(den[64:65, :], avs[h][64:65, :])
                        nc.sync.dma_start(out=dfold[:, 4 * h:4 * h + 4], in_=den[64:65, :])
                        # evacuate the numerator too so the PSUM bank frees
                        # without waiting for the normalization chain
                        araw = nrm_work.tile([64, 512], bf16, tag="araw", bufs=4)
                        nc.vector.tensor_copy(araw[:], avs[h][0:64, :])
                        araws.append(araw)
                    nc.vector.reciprocal(dfold[:], dfold[:])
                    for h in range(NHC):
                        rc0 = nrm_work.tile([1, 512], f32, tag="rc0")
                        nc.sync.dma_start(out=rc0[:], in_=dfold[:, 4 * h:4 * h + 4])
                        bcs = nrm_work.tile([64, 512], f32, tag="bcs")
                        nc.gpsimd.partition_broadcast(bcs[:], rc0[:])
                        pair, odd = divmod(h, 2)
                        if odd:
                            om = nrm_work.tile([64, 512], bf16, tag="om")
                            nc.vector.tensor_mul(om[:], araws[h][:], bcs[:])
                            nc.sync.dma_start(out=avt[64:128, pair, qs], in_=om[:])
                        else:
                            nc.vector.tensor_mul(avt[0:64, pair, qs], araws[h][:], bcs[:])

            # ---- phase 3: output projection (head pairs, K=128) ----
            oout_sb = ctx.enter_context(tc.tile_pool(name="oout", bufs=4))
            with tc.tile_pool(name="opp", bufs=4, space="PSUM") as opp:
                for qi in range(NS):
                    for nch in range(NQ):
                        op = opp.tile([128, 512], f32, tag="op")
                        for p in range(2):
                            nc.tensor.matmul(op[:],
                                             lhsT=avt[:, p, qi * 128:(qi + 1) * 128],
                                             rhs=wo_sb[:, p, nch * 512:(nch + 1) * 512],
                                             start=(p == 0), stop=(p == 1))
                        ot = oout_sb.tile([128, 512], f32, tag="ot")
                        nc.vector.tensor_copy(ot[:], op[:])
                        nc.sync.dma_start(
                            out=out[qi * 128:(qi + 1) * 128, nch * 512:(nch + 1) * 512],
                            in_=ot[:])
    return nc


def _get_nc():
    global _CACHED_NC
    if _CACHED_NC is None:
        nc = _build_nc()
        if not nc.is_finalized():
            nc.finalize()
        _CACHED_NC = nc
    return _CACHED_NC


def _prep_in_maps(hidden_states, Wq, Wk, Wv, Wo, lambda_param):
    lam = math.tanh(math.log1p(math.exp(float(lambda_param))))
    scale = HD ** -0.5
    in_maps = []
    hTb = [np.ascontiguousarray(hidden_states[b].T).astype(BF) for b in range(2)]
    for core in range(8):
        b, g = divmod(core, 4)
        heads = range(NHC * g, NHC * g + NHC)
        wq_cols, wk_cols = [], []
        for h in heads:
            wq_cols.append(Wq[:, h * 64:(h + 1) * 64] * scale)
            wq_cols.append(Wq[:, (NH + h) * 64:(NH + h + 1) * 64] * (-lam * scale))
            wk_cols.append(Wk[:, h * 64:(h + 1) * 64])
            wk_cols.append(Wk[:, (NH + h) * 64:(NH + h + 1) * 64])
        wv_pad = np.zeros((H, NHC * 65), dtype=np.float32)
        for j, h in enumerate(heads):
            wv_pad[:, j * 65:j * 65 + 64] = Wv[:, h * 64:(h + 1) * 64]
        heads = list(heads)
        wo_sel = np.zeros((128, 2, S), dtype=np.float32)  # head-pair stacked rows
        for p in range(2):
            h0, h1 = heads[2 * p], heads[2 * p + 1]
            wo_sel[0:64, p] = Wo[h0 * 64:(h0 + 1) * 64, :]
            wo_sel[64:128, p] = Wo[h1 * 64:(h1 + 1) * 64, :]
        in_maps.append({
            "hT": hTb[b],
            "wq": np.concatenate(wq_cols, axis=1).astype(BF),
            "wk": np.concatenate(wk_cols, axis=1).astype(BF),
            "wv": wv_pad.astype(BF),
            "wo": np.ascontiguousarray(wo_sel).astype(BF),
        })
    return in_maps


def _mask_is_causal(attention_mask):
    m = np.asarray(attention_mask)
    if m.shape != (2, 1, S, S):
        return False
    neg = np.float32(np.finfo(np.float32).min)
    # sampled structural check + full verification
    tri = np.tril(np.ones((S, S), dtype=bool))
    expect = np.where(tri, np.float32(0.0), neg)
    return all(np.array_equal(m[b, 0], expect) for b in range(m.shape[0]))


def _fallback(hidden_states, attention_mask, Wq, Wk, Wv, Wo, lambda_param):
    hs = hidden_states.astype(np.float32)
    lam = math.tanh(math.log1p(math.exp(float(lambda_param))))
    scaling = HD ** -0.5
    B = hs.shape[0]
    out = np.empty((B, S, H), dtype=np.float32)
    for b in range(B):
        q_all = (hs[b] @ Wq).reshape(S, 2 * NH, HD).transpose(1, 0, 2)
        k_all = (hs[b] @ Wk).reshape(S, 2 * NH, HD).transpose(1, 0, 2)
        v = (hs[b] @ Wv).reshape(S, NH, HD).transpose(1, 0, 2)
        acc = np.zeros((S, H), dtype=np.float32)
        for h in range(NH):
            s1 = q_all[h] @ k_all[h].T
            s2 = q_all[NH + h] @ k_all[NH + h].T
            sc = (s1 - lam * s2) * scaling + attention_mask[b, 0]
            sc -= sc.max(axis=-1, keepdims=True)
            p = np.exp(sc)
            p /= p.sum(axis=-1, keepdims=True)
            acc += (p @ v[h]) @ Wo[h * 64:(h + 1) * 64]
        out[b] = acc
    return out


def _run(inputs, trace=False):
    from concourse.bass_utils import run_bass_kernel_spmd

    hidden_states = np.asarray(inputs["hidden_states"], dtype=np.float32)
    attention_mask = np.asarray(inputs["attention_mask"], dtype=np.float32)
    Wq = np.asarray(inputs["Wq"], dtype=np.float32)
    Wk = np.asarray(inputs["Wk"], dtype=np.float32)
    Wv = np.asarray(inputs["Wv"], dtype=np.float32)
    Wo = np.asarray(inputs["Wo"], dtype=np.float32)
    lam_p = inputs["lambda_param"]

    if not _mask_is_causal(attention_mask):
        return _fallback(hidden_states, attention_mask, Wq, Wk, Wv, Wo, lam_p), None

    in_maps = _prep_in_maps(hidden_states, Wq, Wk, Wv, Wo, lam_p)
    nc = _get_nc()
    res = run_bass_kernel_spmd(nc, in_maps, list(range(8)), trace=trace)
    out = np.empty((2, S, H), dtype=np.float32)
    for b in range(2):
        acc = res.results[4 * b]["out"].astype(np.float32)
        for g in range(1, 4):
            acc = acc + res.results[4 * b + g]["out"]
        out[b] = acc
    return out, res


def kernel(**inputs):
    out, _ = _run(inputs, trace=False)
    return out


# revision 18
# speedup vs baseline: 1.1624x; 1.0078x over previous
"""Differential attention kernel for 8 Trainium2 NeuronCores.

Sharding: batch x head-group. Core c handles batch b = c//4, heads
[4g, 4g+4) with g = c%4. Each core computes Q/K/V projections for its
heads over the full sequence, causal differential attention, and its
partial O-projection; the host sums the 4 partials per batch.

Differential attention trick: score = (q1.k1 - lam*q2.k2) * scale is a
single K=128 matmul with stacked [q1*scale; -lam*scale*q2] and [k1; k2]
head vectors (scales folded into the projection weights on the host).

Softmax: scores are computed transposed (keys on partitions, queries
free), exp'd without max subtraction (inputs are bounded; exp is exact
to 2ULP on ACT), and the denominator comes for free from a ones-column
appended to V in the P@V matmul. Causality is applied structurally
(upper blocks skipped, diagonal blocks zeroed post-exp), which the host
validates against the attention_mask input before dispatch.
"""
import math
from contextlib import ExitStack

import numpy as np
import ml_dtypes

S = 2048
H = 2048
NH = 16
HD = 64
NHC = 4          # heads per core
BF = ml_dtypes.bfloat16

_CACHED_NC = None


def _build_nc():
    import concourse.mybir as mybir
    import concourse.tile as tile
    from concourse import bacc

    bf16 = mybir.dt.bfloat16
    f32 = mybir.dt.float32
    AF = mybir.ActivationFunctionType

    nc = bacc.Bacc(None, target_bir_lowering=False)
    hT = nc.declare_dram_parameter("hT", [H, S], bf16, isOutput=False)
    wq = nc.declare_dram_parameter("wq", [H, NHC * 128], bf16, isOutput=False)
    wk = nc.declare_dram_parameter("wk", [H, NHC * 128], bf16, isOutput=False)
    wv = nc.declare_dram_parameter("wv", [H, NHC * 65], bf16, isOutput=False)
    wo = nc.declare_dram_parameter("wo", [128, 2, S], bf16, isOutput=False)
    out = nc.declare_dram_parameter("out", [S, H], f32, isOutput=True)

    KT = H // 128    # 16 contraction tiles for projections
    NQ = S // 512    # 4 query chunks
    NS = S // 128    # 16 seq tiles

    with tile.TileContext(nc) as tc:
        with ExitStack() as ctx:
            # ---- persistent SBUF ----
            sb = ctx.enter_context(tc.tile_pool(name="sb", bufs=1))
            qk_sb = ctx.enter_context(tc.tile_pool(name="qk", bufs=1))
            ht_all = sb.tile([128, KT, S], bf16)          # hidden^T
            wq_sb = sb.tile([128, KT, NHC * 128], bf16)
            wk_sb = sb.tile([128, KT, NHC * 128], bf16)
            wv_sb = sb.tile([128, KT, NHC * 65], bf16)
            wo_sb = sb.tile([128, 2, S], bf16)            # head-pair stacked Wo rows
            qT = qk_sb.tile([128, NHC, S], bf16)          # [q1*s; -lam*s*q2] per head
            kT = qk_sb.tile([128, NHC, S], bf16)          # [k1; k2] per head
            v4 = qk_sb.tile([128, NS, NHC * 65], bf16)    # V tiles + ones cols
            avt = qk_sb.tile([128, 2, S], bf16)           # attn_out^T, head pairs stacked
            # warm the ACT exp table while DMAs stream in
            warm = sb.tile([1, 16], f32)
            nc.vector.memset(warm[:], 0.0)
            nc.scalar.activation(warm[:], warm[:], AF.Exp)

            # ---- input DMAs: k-ordered contiguous rows so projection
            # chains (k-outer) can start as soon as the first rows land ----
            for k in range(KT):
                nc.sync.dma_start(out=ht_all[:, k, :], in_=hT[k * 128:(k + 1) * 128, :])
                nc.sync.dma_start(out=wq_sb[:, k, :], in_=wq[k * 128:(k + 1) * 128, :])
                nc.sync.dma_start(out=wk_sb[:, k, :], in_=wk[k * 128:(k + 1) * 128, :])
                nc.sync.dma_start(out=wv_sb[:, k, :], in_=wv[k * 128:(k + 1) * 128, :])
            nc.sync.dma_start(out=wo_sb[:], in_=wo[:, :, :])

            # ---- phase 1a: q/k projections (k-outer, nj-inner) ----
            with tc.tile_pool(name="pjp", bufs=2, space="PSUM") as pjp:
                for h in range(NHC):
                    hs = slice(h * 128, (h + 1) * 128)
                    for w_sb, dst in ((wq_sb, qT), (wk_sb, kT)):
                        pp = pjp.tile([128, S], f32, tag="qkp")
                        for k in range(KT):
                            for nj in range(NQ):
                                nc.tensor.matmul(pp[:, nj * 512:(nj + 1) * 512],
                                                 lhsT=w_sb[:, k, hs],
                                                 rhs=ht_all[:, k, nj * 512:(nj + 1) * 512],
                                                 start=(k == 0), stop=(k == KT - 1))
                        for nj in range(NQ):
                            nc.vector.tensor_copy(dst[:, h, nj * 512:(nj + 1) * 512],
                                                  pp[:, nj * 512:(nj + 1) * 512])

            # ---- phase 1b: V projection (natural [S, 260] layout) ----
            with tc.tile_pool(name="vjp", bufs=3, space="PSUM") as vjp:
                for st in range(NS):
                    vp = vjp.tile([128, NHC * 65], f32, tag="vp")
                    for k in range(KT):
                        nc.tensor.matmul(vp[:], lhsT=ht_all[:, k, st * 128:(st + 1) * 128],
                                         rhs=wv_sb[:, k, :],
                                         start=(k == 0), stop=(k == KT - 1))
                    nc.vector.tensor_copy(v4[:, st, :], vp[:])
                    for j in range(NHC):
                        nc.gpsimd.memset(v4[:, st, j * 65 + 64:j * 65 + 65], 1.0)

            # ---- phase 2: attention ----
            att_work = ctx.enter_context(tc.tile_pool(name="attw", bufs=3))
            nrm_work = ctx.enter_context(tc.tile_pool(name="nrmw", bufs=2))
            with tc.tile_pool(name="atp", bufs=2, space="PSUM") as atp:
                for nj in range(NQ):
                    qs = slice(nj * 512, (nj + 1) * 512)
                    nblk = 4 * nj + 4
                    avs = []
                    for hp in range(2):
                        pair_heads = (2 * hp, 2 * hp + 1)
                        pav = {}
                        for h in pair_heads:
                            pav[h] = atp.tile([65, 512], f32, tag="av", bufs=4, name=f"av{h}")
                            avs.append(pav[h])
                        # round-robin the two heads per key-block group so one
                        # head's score matmuls hide the other head's exp
                        for kg in range(nblk // 2):
                            scs, pts = {}, {}
                            g0 = 256 if kg == 2 * nj + 1 else 0
                            for h in pair_heads:
                                sc = atp.tile([128, 1024], f32, tag="sc", name=f"sc{h}")
                                scs[h] = sc
                                for u in range(2):
                                    ki = 2 * kg + u
                                    nc.tensor.matmul(sc[:, u * 512:(u + 1) * 512],
                                                     lhsT=kT[:, h, ki * 128:(ki + 1) * 128],
                                                     rhs=qT[:, h, qs], start=True, stop=True)
                            for h in pair_heads:
                                pt = att_work.tile([128, 1024], bf16, tag="pt", bufs=4, name=f"pt{h}")
                                pts[h] = pt
                                # columns below 128*uu of a diagonal block are
                                # fully masked; skip in exp and the P@V matmul
                                nc.scalar.activation(pt[:, g0:1024], scs[h][:, g0:1024], AF.Exp)
                            for h in pair_heads:
                                pt, av = pts[h], pav[h]
                                for u in range(2):
                                    ki = 2 * kg + u
                                    uu = ki - 4 * nj  # >=0 on diagonal blocks
                                    if uu >= 0:
                                        b0 = u * 512 + uu * 128
                                        nc.gpsimd.affine_select(
                                            out=pt[:, b0:b0 + 128],
                                            in_=pt[:, b0:b0 + 128],
                                            compare_op=mybir.AluOpType.is_ge,
                                            fill=0.0,
                                            base=0,
                                            channel_multiplier=-1,
                                            pattern=[[1, 128]],
                                        )
                                        nc.tensor.matmul(av[:, uu * 128:512],
                                                         lhsT=v4[:, ki, h * 65:(h + 1) * 65],
                                                         rhs=pt[:, u * 512 + uu * 128:(u + 1) * 512],
                                                         start=(ki == 0), stop=(ki == nblk - 1))
                                    else:
                                        nc.tensor.matmul(av[:],
                                                         lhsT=v4[:, ki, h * 65:(h + 1) * 65],
                                                         rhs=pt[:, u * 512:(u + 1) * 512],
                                                         start=(ki == 0), stop=(ki == nblk - 1))
                    # normalize after all heads of this chunk: row 64 of av
                    # is the softmax denominator; scale rows 0-63 by 1/denom.
                    # Reciprocal runs on a [128, 16] DMA-folded layout so all
                    # 128 DVE lanes work instead of one.
                    dfold = nrm_work.tile([128, 16], f32, tag="dfold")
                    araws = []
                    for h in range(NHC):
                        den = nrm_work.tile([65, 512], f32, tag="den", bufs=4)
                        nc.vector.tensor_copy(den[64:65, :], avs[h][64:65, :])
                        nc.sync.dma_start(out=dfold[:, 4 * h:4 * h + 4], in_=den[64:65, :])
                        # evacuate the numerator too so the PSUM bank frees
                        # without waiting for the normalization chain
                        araw = nrm_work.tile([64, 512], bf16, tag="araw", bufs=4)
                        nc.vector.tensor_copy(araw[:], avs[h][0:64, :])
                        araws.append(araw)
                    nc.vector.reciprocal(dfold[:], dfold[:])
                    for h in range(NHC):
                        rc0 = nrm_work.tile([1, 512], f32, tag="rc0")
                        nc.sync.dma_start(out=rc0[:], in_=dfold[:, 4 * h:4 * h + 4])
                        bcs = nrm_work.tile([64, 512], f32, tag="bcs")
                        nc.gpsimd.partition_broadcast(bcs[:], rc0[:])
                        pair, odd = divmod(h, 2)
                        if odd:
                            om = nrm_work.tile([64, 512], bf16, tag="om")
                            nc.vector.tensor_mul(om[:], araws[h][:], bcs[:])
                            nc.sync.dma_start(out=avt[64:128, pair, qs], in_=om[:])
                        else:
                            nc.vector.tensor_mul(avt[0:64, pair, qs], araws[h][:], bcs[:])

            # ---- phase 3: output projection (head pairs, K=128) ----
            oout_sb = ctx.enter_context(tc.tile_pool(name="oout", bufs=4))
            with tc.tile_pool(name="opp", bufs=4, space="PSUM") as opp:
                for qi in range(NS):
                    for nch in range(NQ):
                        op = opp.tile([128, 512], f32, tag="op")
                        for p in range(2):
                            nc.tensor.matmul(op[:],
                                             lhsT=avt[:, p, qi * 128:(qi + 1) * 128],
                                             rhs=wo_sb[:, p, nch * 512:(nch + 1) * 512],
                                             start=(p == 0), stop=(p == 1))
                        ot = oout_sb.tile([128, 512], f32, tag="ot")
                        nc.vector.tensor_copy(ot[:], op[:])
                        nc.sync.dma_start(
                            out=out[qi * 128:(qi + 1) * 128, nch * 512:(nch + 1) * 512],
                            in_=ot[:])
    return nc


def _get_nc():
    global _CACHED_NC
    if _CACHED_NC is None:
        nc = _build_nc()
        if not nc.is_finalized():
            nc.finalize()
        _CACHED_NC = nc
    return _CACHED_NC


def _prep_in_maps(hidden_states, Wq, Wk, Wv, Wo, lambda_param):
    lam = math.tanh(math.log1p(math.exp(float(lambda_param))))
    scale = HD ** -0.5
    in_maps = []
    hTb = [np.ascontiguousarray(hidden_states[b].T).astype(BF) for b in range(2)]
    for core in range(8):
        b, g = divmod(core, 4)
        heads = range(NHC * g, NHC * g + NHC)
        wq_cols, wk_cols = [], []
        for h in heads:
            wq_cols.append(Wq[:, h * 64:(h + 1) * 64] * scale)
            wq_cols.append(Wq[:, (NH + h) * 64:(NH + h + 1) * 64] * (-lam * scale))
            wk_cols.append(Wk[:, h * 64:(h + 1) * 64])
            wk_cols.append(Wk[:, (NH + h) * 64:(NH + h + 1) * 64])
        wv_pad = np.zeros((H, NHC * 65), dtype=np.float32)
        for j, h in enumerate(heads):
            wv_pad[:, j * 65:j * 65 + 64] = Wv[:, h * 64:(h + 1) * 64]
        heads = list(heads)
        wo_sel = np.zeros((128, 2, S), dtype=np.float32)  # head-pair stacked rows
        for p in range(2):
            h0, h1 = heads[2 * p], heads[2 * p + 1]
            wo_sel[0:64, p] = Wo[h0 * 64:(h0 + 1) * 64, :]
            wo_sel[64:128, p] = Wo[h1 * 64:(h1 + 1) * 64, :]
        in_maps.append({
            "hT": hTb[b],
            "wq": np.concatenate(wq_cols, axis=1).astype(BF),
            "wk": np.concatenate(wk_cols, axis=1).astype(BF),
            "wv": wv_pad.astype(BF),
            "wo": np.ascontiguousarray(wo_sel).astype(BF),
        })
    return in_maps


def _mask_is_causal(attention_mask):
    m = np.asarray(attention_mask)
    if m.shape != (2, 1, S, S):
        return False
    neg = np.float32(np.finfo(np.float32).min)
    # sampled structural check + full verification
    tri = np.tril(np.ones((S, S), dtype=bool))
    expect = np.where(tri, np.float32(0.0), neg)
    return all(np.array_equal(m[b, 0], expect) for b in range(m.shape[0]))


def _fallback(hidden_states, attention_mask, Wq, Wk, Wv, Wo, lambda_param):
    hs = hidden_states.astype(np.float32)
    lam = math.tanh(math.log1p(math.exp(float(lambda_param))))
    scaling = HD ** -0.5
    B = hs.shape[0]
    out = np.empty((B, S, H), dtype=np.float32)
    for b in range(B):
        q_all = (hs[b] @ Wq).reshape(S, 2 * NH, HD).transpose(1, 0, 2)
        k_all = (hs[b] @ Wk).reshape(S, 2 * NH, HD).transpose(1, 0, 2)
        v = (hs[b] @ Wv).reshape(S, NH, HD).transpose(1, 0, 2)
        acc = np.zeros((S, H), dtype=np.float32)
        for h in range(NH):
            s1 = q_all[h] @ k_all[h].T
            s2 = q_all[NH + h] @ k_all[NH + h].T
            sc = (s1 - lam * s2) * scaling + attention_mask[b, 0]
            sc -= sc.max(axis=-1, keepdims=True)
            p = np.exp(sc)
            p /= p.sum(axis=-1, keepdims=True)
            acc += (p @ v[h]) @ Wo[h * 64:(h + 1) * 64]
        out[b] = acc
    return out


def _run(inputs, trace=False):
    from concourse.bass_utils import run_bass_kernel_spmd

    hidden_states = np.asarray(inputs["hidden_states"], dtype=np.float32)
    attention_mask = np.asarray(inputs["attention_mask"], dtype=np.float32)
    Wq = np.asarray(inputs["Wq"], dtype=np.float32)
    Wk = np.asarray(inputs["Wk"], dtype=np.float32)
    Wv = np.asarray(inputs["Wv"], dtype=np.float32)
    Wo = np.asarray(inputs["Wo"], dtype=np.float32)
    lam_p = inputs["lambda_param"]

    if not _mask_is_causal(attention_mask):
        return _fallback(hidden_states, attention_mask, Wq, Wk, Wv, Wo, lam_p), None

    in_maps = _prep_in_maps(hidden_states, Wq, Wk, Wv, Wo, lam_p)
    nc = _get_nc()
    res = run_bass_kernel_spmd(nc, in_maps, list(range(8)), trace=trace)
    out = np.empty((2, S, H), dtype=np.float32)
    for b in range(2):
        acc = res.results[4 * b]["out"].astype(np.float32)
        for g in range(1, 4):
            acc = acc + res.results[4 * b + g]["out"]
        out[b] = acc
    return out, res


def kernel(**inputs):
    out, _ = _run(inputs, trace=False)
    return out
